# revision 20
# baseline (speedup 1.0000x reference)
"""Trainium2 Bass kernel for nn_GainAKT4 (dual-stream knowledge-tracing transformer).

Strategy
--------
Data-parallel over batch: B=32 -> 4 batch elements per core x 8 cores.
Params replicated on every core (weights resident in SBUF; mlp1_W2 streamed).

Per-core layout conventions (per batch element b):
  S-layout ("token-major"): [128 tokens(part), chunk j, feat]   - for LayerNorm,
      softmax row-normalisation, residuals (per-token scalars = per-partition).
  T-layout ("feature-major"): [128 feats(part), chunk, 512 tokens] - for the
      matmul chain: Y^T = W^T X^T via matmul(lhsT=W[K,M], rhs=X^T[K,N]).
      Linear-layer biases become per-partition scalars -> fused into the
      PSUM->SBUF eviction on the Scalar engine (Relu/Exp/Identity + bias AP).

Attention: scores are built transposed (PT[k,q] = scores^T) straight from
Q^T/K^T; no max-subtraction (scores are provably tiny); causal handled by
computing only k<=q blocks + a triangular 0/1 mask on diagonal blocks; the
softmax denominator falls out of an extra ones-column in V during the
attn@V matmul; normalisation happens on the S-layout output where 1/sum is a
per-partition scalar.

Mastery head: kc = clip(cumsum(exp(li-2)),0,1) runs in T-layout so the L-axis
cumsum is a single VectorE tensor_tensor_scan (op0=add, op1=min vs ones ==
fused cumsum+clip, exact because increments are positive). kc is written to
DRAM as [2000, 512] per batch element (contiguous rows); the host transposes
on unshard. take_along_axis is a flat-index indirect DMA gather from the kc
DRAM buffer (one element per partition).
"""

import os
import sys
import numpy as np
from contextlib import ExitStack

for _p in ("/opt/trn_rl_repo",):
    if _p not in sys.path and os.path.isdir(_p):
        sys.path.insert(0, _p)

import concourse.bass as bass
import concourse.tile as tile
from concourse import bacc, mybir
from concourse import bass_utils
from concourse.masks import make_identity, make_upper_triangular
from concourse.tile_rust import add_dep_helper

F32 = mybir.dt.float32
F32R = mybir.dt.float32r
I32 = mybir.dt.int32
AF = mybir.ActivationFunctionType
OP = mybir.AluOpType

# problem constants
NUM_C = 2000
L = 512
D = 256
H = 4
DK = 64
NB = 4
DFF = 512
B = 32
NCORES = 8
BL = B // NCORES          # batch elements per core
P = 128
NJ = L // P               # token chunks (4)
NF = D // P               # feature chunks (2)
NFF = DFF // P            # dff chunks (4)
EPS = 1e-5
# channel chunks for the 2000-wide mastery head
CCH = []
_c0 = 0
while _c0 < NUM_C:
    CCH.append((_c0, min(P, NUM_C - _c0)))
    _c0 += P
NCC = len(CCH)            # 16 (15x128 + 80)

MM_DT = os.environ.get("KMM_DT", "f32")   # "f32" | "f32r"


WDT = F32R if MM_DT == "f32r" else F32   # dtype of matmul-feeding tiles
ADT = F32R   # attention + mlp1_W2 matmuls always run in f32r (error-tolerant)


def _mm(ap):
    return ap


# --------------------------------------------------------------------------
# program build
# --------------------------------------------------------------------------

def _build_program(flags):
    """flags: (ln_identity, bo_zero, f2b_zero) -> (nc, input_names)"""
    ln_identity, bo_zero, f2b_zero = flags
    nc = bacc.Bacc(
        "TRN2",
        target_bir_lowering=False,
        debug=False,
        enable_asserts=False,
        num_devices=1,
    )

    dram = {}

    def din(name, shape, dtype=F32):
        dram[name] = nc.dram_tensor(name, list(shape), dtype, kind="ExternalInput")
        return dram[name]

    def dout(name, shape, dtype=F32):
        dram[name] = nc.dram_tensor(name, list(shape), dtype, kind="ExternalOutput")
        return dram[name]

    # ---- inputs ----
    din("tok_idx", (BL, L, 1), I32)
    din("qry_idx", (BL, L, 1), I32)
    din("ssk_idx", (BL, L, 1), I32)
    din("pos_S", (P, NJ, D))
    din("ctx_emb", (2 * NUM_C, D))
    din("val_emb", (2 * NUM_C, D))
    din("skill_emb", (NUM_C, D))
    for i in range(NB):
        din(f"Wq_{i}", (D, D), WDT)
        din(f"Wk_{i}", (D, D), WDT)
        din(f"Wv_{i}", (D, D), WDT)
        din(f"Wo_{i}", (D, D), WDT)
        din(f"f1W_{i}", (D, DFF), WDT)
        din(f"f2W_{i}", (DFF, D), WDT)
        din(f"bq_{i}", (P, NF))
        din(f"bk_{i}", (P, NF))
        din(f"f1b_{i}", (P, NFF))
        if not bo_zero:
            din(f"bo_{i}", (1, D))       # bv@Wo + bo, broadcast on device
        if not f2b_zero:
            din(f"f2b_{i}", (1, D))
        if not ln_identity:
            for nm in ("ln1c", "ln1v", "ln2c"):
                din(f"{nm}g_{i}", (1, D))
                din(f"{nm}b_{i}", (1, D))
    din("head_W1", (3 * D, DFF), WDT)
    din("head_b1", (P, NFF))
    din("head_W2", (DFF, D), WDT)
    din("head_b2", (P, NF))
    din("head_W3", (D, 1), WDT)
    din("head_b3", (1, 1))
    din("mlp1_W1", (D, DFF), WDT)
    din("mlp1_b1", (P, NFF))
    din("mlp1_W2", (DFF, NUM_C), ADT)
    din("b2m2", (P, NCC))               # mlp1_b2 - 2.0, column-chunked
    din("mlp2_W1", (2 * D + 1, DFF), WDT)
    din("mlp2_b1", (P, NFF))
    din("mlp2_W2", (DFF, D), WDT)
    din("mlp2_b2", (P, NF))
    din("mlp2_W3", (D, 1), WDT)
    din("mlp2_b3", (1, 1))

    # ---- outputs ----
    dout("o_bce", (BL, L))
    dout("o_logits", (BL, L))
    dout("o_mast", (BL, L))
    dout("o_mlogits", (BL, L))
    kc_outs = [dout(f"o_kc_{b}", (NUM_C, L)) for b in range(BL)]

    with tile.TileContext(nc) as tc, ExitStack() as ctx:
        wpool = ctx.enter_context(tc.tile_pool(name="weights", bufs=1))
        cpool = ctx.enter_context(tc.tile_pool(name="consts", bufs=1))
        actp = ctx.enter_context(tc.tile_pool(name="acts", bufs=1))
        bigp = ctx.enter_context(tc.tile_pool(name="bigacts", bufs=1))
        statp = ctx.enter_context(tc.tile_pool(name="stats", bufs=4))
        tinyp = ctx.enter_context(tc.tile_pool(name="tiny", bufs=8))
        idxp = ctx.enter_context(tc.tile_pool(name="idx", bufs=4))
        rowp = ctx.enter_context(tc.tile_pool(name="rows", bufs=3))
        kcp = ctx.enter_context(tc.tile_pool(name="kc", bufs=3))
        w2p = ctx.enter_context(tc.tile_pool(name="w2s", bufs=3))
        psq = ctx.enter_context(tc.tile_pool(name="psq", bufs=3, space="PSUM"))
        pss = ctx.enter_context(tc.tile_pool(name="pss", bufs=3, space="PSUM"))
        pstp = ctx.enter_context(tc.tile_pool(name="pst", bufs=2, space="PSUM"))

        # ---------------- constants & weights (once) ----------------
        identity = cpool.tile([P, P], F32, tag="identity")
        make_identity(nc, identity[:])
        triu = cpool.tile([P, P], F32, tag="triu")
        make_upper_triangular(nc, triu[:], val=1.0, diag=True)  # 1 where k<=q
        ones512 = cpool.tile([P, L], F32, tag="ones512")
        nc.vector.memset(ones512[:], 1.0)
        ones_row = cpool.tile([1, P], F32, tag="ones_row")
        nc.vector.memset(ones_row[:], 1.0)
        eps_t = cpool.tile([P, 1], F32, tag="eps_t")
        nc.vector.memset(eps_t[:], EPS)

        pos_S = cpool.tile([P, NJ, D], F32, tag="pos_S")
        nc.sync.dma_start(pos_S[:], dram["pos_S"].ap())

        wt = {}

        def _round_w(t):
            if t.dtype == F32R:
                nc.vector.tensor_copy(t[:], t[:].bitcast(F32))

        def loadw(name, K, M):
            kc_ = K // P
            t = wpool.tile([P, kc_, M], WDT, tag=name)
            nc.sync.dma_start(
                t[:], dram[name].ap().rearrange("(ko p) m -> p ko m", p=P)
            )
            _round_w(t)
            wt[name] = t

        def loadcol(name, ncol):
            t = wpool.tile([P, ncol], F32, tag=name)
            nc.sync.dma_start(t[:], dram[name].ap())
            wt[name] = t

        def bcast_row(name):
            """[1, D] DRAM row -> [128, D] SBUF broadcast tile (via PE)."""
            rt = wpool.tile([1, D], F32, tag=name + "_r")
            nc.sync.dma_start(rt[:], dram[name].ap())
            ps = pss.tile([P, D], F32, tag="psS")
            nc.tensor.matmul(ps[:], lhsT=_mm(ones_row[:]), rhs=_mm(rt[:]),
                             start=True, stop=True)
            t = wpool.tile([P, D], F32, tag=name)
            nc.vector.tensor_copy(t[:], ps[:])
            wt[name] = t

        for i in range(NB):
            for nm in ("Wq", "Wk", "Wv", "Wo"):
                loadw(f"{nm}_{i}", D, D)
            loadw(f"f1W_{i}", D, DFF)
            loadw(f"f2W_{i}", DFF, D)
            loadcol(f"bq_{i}", NF)
            loadcol(f"bk_{i}", NF)
            loadcol(f"f1b_{i}", NFF)
            if not bo_zero:
                bcast_row(f"bo_{i}")
            if not f2b_zero:
                bcast_row(f"f2b_{i}")
            if not ln_identity:
                for nm in ("ln1c", "ln1v", "ln2c"):
                    bcast_row(f"{nm}g_{i}")
                    bcast_row(f"{nm}b_{i}")
        loadw("head_W1", 3 * D, DFF)
        loadw("head_W2", DFF, D)
        loadw("head_W3", D, 1)
        loadw("mlp1_W1", D, DFF)
        loadw("mlp2_W2", DFF, D)
        loadw("mlp2_W3", D, 1)
        loadcol("head_b1", NFF)
        loadcol("head_b2", NF)
        loadcol("mlp1_b1", NFF)
        loadcol("b2m2", NCC)
        loadcol("mlp2_b1", NFF)
        loadcol("mlp2_b2", NF)
        # mlp2_W1: 513 rows = 4x128 + 1
        t = wpool.tile([P, 4, DFF], WDT, tag="mlp2_W1")
        nc.sync.dma_start(
            t[:], dram["mlp2_W1"].ap()[0:512, :].rearrange("(ko p) m -> p ko m", p=P)
        )
        _round_w(t)
        wt["mlp2_W1"] = t
        t = wpool.tile([1, DFF], WDT, tag="mlp2_w1row")
        nc.sync.dma_start(t[:], dram["mlp2_W1"].ap()[512:513, :])
        _round_w(t)
        wt["mlp2_w1row"] = t
        for nm in ("head_b3", "mlp2_b3"):
            t = wpool.tile([1, 1], F32, tag=nm)
            nc.sync.dma_start(t[:], dram[nm].ap())
            wt[nm] = t

        # ---------------- helpers ----------------
        def transpose_S_to_T(src_S, tag):
            dst = actp.tile([P, NF, L], WDT, tag=tag)
            for j in range(NJ):
                for fc in range(NF):
                    pt = pstp.tile([P, P], F32, tag="tp")
                    nc.tensor.transpose(
                        pt[:], src_S[:, j, fc * P:(fc + 1) * P], identity[:]
                    )
                    nc.vector.tensor_copy(dst[:, fc, j * P:(j + 1) * P], pt[:])
            return dst

        def linT(xT_chunks, wname, bname, act, tag, mchunks, out_pool=None, dt=None):
            """Y^T = act(W^T X^T + b). xT_chunks: list of [128, 512] APs (K chunks)."""
            w = wt[wname]
            pool = out_pool or actp
            out = pool.tile([P, mchunks, L], dt or WDT, tag=tag)
            nk = len(xT_chunks)
            for m in range(mchunks):
                ps = psq.tile([P, L], F32, tag="mm512")
                for k in range(nk):
                    nc.tensor.matmul(
                        ps[:], lhsT=_mm(w[:, k, m * P:(m + 1) * P]),
                        rhs=_mm(xT_chunks[k]),
                        start=(k == 0), stop=(k == nk - 1),
                    )
                bcol = wt[bname][:, m:m + 1]
                if act == "relu":
                    nc.scalar.activation(out[:, m, :], ps[:], AF.Relu, bias=bcol)
                else:  # plain bias add on DVE
                    nc.vector.tensor_scalar(
                        out[:, m, :], ps[:], bcol, None, op0=OP.add
                    )
            return out

        def layer_norm(r_list, stats, s0, n, out_S_list, g=None, bvec=None):
            """r_list[j]: [128, D] residual APs; stats cols [s0, s0+n) = sums,
            [s0+8, s0+8+n) = sumsq. Writes normalised result to out_S_list[j]."""
            work = statp.tile([P, 8], F32, tag="lnwork")
            mean = work[:, 0:n]
            nc.vector.tensor_scalar_mul(mean, stats[:, s0:s0 + n], 1.0 / D)
            msq = work[:, 4:4 + n]
            nc.vector.tensor_tensor(msq, mean, mean, op=OP.mult)
            var_t = statp.tile([P, 8], F32, tag="lnvar")
            var = var_t[:, 0:n]
            nc.vector.scalar_tensor_tensor(
                var, stats[:, s0 + 8:s0 + 8 + n], 1.0 / D, msq,
                op0=OP.mult, op1=OP.subtract,
            )
            std = statp.tile([P, 8], F32, tag="lnstd")
            nc.scalar.activation(std[:, 0:n], var, AF.Sqrt, bias=eps_t[:, 0:1])
            rstd = std[:, 4:4 + n]
            nc.vector.reciprocal(rstd, std[:, 0:n])
            nmr_t = statp.tile([P, 8], F32, tag="lnnmr")
            nmr = nmr_t[:, 0:n]
            nc.vector.scalar_tensor_tensor(
                nmr, mean, -1.0, rstd, op0=OP.mult, op1=OP.mult
            )
            for j in range(n):
                nc.scalar.activation(
                    out_S_list[j], r_list[j], AF.Identity,
                    bias=nmr[:, j:j + 1], scale=rstd[:, j:j + 1],
                )
                if g is not None:
                    nc.vector.tensor_tensor(out_S_list[j], out_S_list[j], g[:], op=OP.mult)
                    nc.vector.tensor_tensor(out_S_list[j], out_S_list[j], bvec[:], op=OP.add)

        # ---------------- per-batch-element pipeline ----------------
        for b in range(BL):
            # --- embedding gather + pos ---
            ctx_S = actp.tile([P, NJ, D], F32, tag="ctx_S")
            val_S = actp.tile([P, NJ, D], F32, tag="val_S")
            for j in range(NJ):
                ti = idxp.tile([P, 1], I32, tag="tok")
                nc.sync.dma_start(ti[:], dram["tok_idx"].ap()[b, j * P:(j + 1) * P, :])
                nc.gpsimd.indirect_dma_start(
                    out=ctx_S[:, j, :], out_offset=None,
                    in_=dram["ctx_emb"].ap(),
                    in_offset=bass.IndirectOffsetOnAxis(ap=ti[:, :1], axis=0),
                )
                nc.gpsimd.indirect_dma_start(
                    out=val_S[:, j, :], out_offset=None,
                    in_=dram["val_emb"].ap(),
                    in_offset=bass.IndirectOffsetOnAxis(ap=ti[:, :1], axis=0),
                )
                nc.vector.tensor_add(ctx_S[:, j, :], ctx_S[:, j, :], pos_S[:, j, :])
                nc.vector.tensor_add(val_S[:, j, :], val_S[:, j, :], pos_S[:, j, :])
            ctx_T = transpose_S_to_T(ctx_S, "ctx_T")
            val_T = transpose_S_to_T(val_S, "val_T")

            # --- encoder blocks ---
            for i in range(NB):
                ctxTch = [ctx_T[:, k, :] for k in range(NF)]
                QT = linT(ctxTch, f"Wq_{i}", f"bq_{i}", "bias", "QT", NF, dt=ADT)
                KT = linT(ctxTch, f"Wk_{i}", f"bk_{i}", "bias", "KT", NF, dt=ADT)

                # V in S-layout with an extra ones column per head
                V_S = actp.tile([P, NJ, H, DK + 2], ADT, tag="V_S")
                wv = wt[f"Wv_{i}"]
                for j in range(NJ):
                    pv = pss.tile([P, D], F32, tag="psS")
                    for k in range(NF):
                        nc.tensor.matmul(
                            pv[:], lhsT=_mm(val_T[:, k, j * P:(j + 1) * P]),
                            rhs=_mm(wv[:, k, :]),
                            start=(k == 0), stop=(k == NF - 1),
                        )
                    nc.vector.tensor_copy(
                        V_S[:, j, :, 0:DK], pv[:].rearrange("p (h d) -> p h d", h=H)
                    )
                    nc.vector.tensor_copy(
                        V_S[:, j, :, DK:DK + 2],
                        ones512[:, 0:2 * H].rearrange("p (h t) -> p h t", h=H),
                    )

                # attention
                attn_S = actp.tile([P, NJ, D], F32, tag="attn_S")
                for h in range(H):
                    hm, hp = h // 2, (h % 2) * DK
                    PT = bigp.tile([P, NJ, L], ADT, tag="PT")
                    for kc in range(NJ):
                        q0 = kc * P
                        ps = psq.tile([P, L], F32, tag="mm512")
                        nc.tensor.matmul(
                            ps[:, q0:], lhsT=_mm(KT[hp:hp + DK, hm, q0:q0 + P]),
                            rhs=_mm(QT[hp:hp + DK, hm, q0:]),
                            start=True, stop=True,
                        )
                        nc.scalar.activation(
                            PT[:, kc, q0:], ps[:, q0:], AF.Exp, scale=0.125
                        )
                        nc.vector.tensor_tensor(
                            PT[:, kc, q0:q0 + P], PT[:, kc, q0:q0 + P], triu[:],
                            op=OP.mult,
                        )
                    for qc in range(NJ):
                        po_t = pss.tile([P, D], F32, tag="psS")
                        po = po_t[:, :DK + 2]
                        for kc in range(qc + 1):
                            nc.tensor.matmul(
                                po, lhsT=_mm(PT[:, kc, qc * P:(qc + 1) * P]),
                                rhs=_mm(V_S[:, kc, h, :]),
                                start=(kc == 0), stop=(kc == qc),
                            )
                        rec = tinyp.tile([P, 1], F32, tag="rec")
                        nc.vector.reciprocal(rec[:], po[:, DK:DK + 1])
                        nc.scalar.activation(
                            attn_S[:, qc, h * DK:(h + 1) * DK], po[:, :DK],
                            AF.Copy, scale=rec[:],
                        )

                attn_T = transpose_S_to_T(attn_S, "attn_T")

                # Wo + residual + LN1c/LN1v
                wo = wt[f"Wo_{i}"]
                rC = actp.tile([P, NJ, D], F32, tag="rC")
                rV = actp.tile([P, NJ, D], F32, tag="rV")
                stats = statp.tile([P, 16], F32, tag="stats")
                for j in range(NJ):
                    pa = pss.tile([P, D], F32, tag="psS")
                    for fc in range(NF):
                        nc.tensor.matmul(
                            pa[:], lhsT=_mm(attn_T[:, fc, j * P:(j + 1) * P]),
                            rhs=_mm(wo[:, fc, :]),
                            start=(fc == 0), stop=(fc == NF - 1),
                        )
                    if not bo_zero:
                        nc.vector.tensor_tensor(pa[:], pa[:], wt[f"bo_{i}"][:], op=OP.add)
                    nc.vector.scalar_tensor_tensor(
                        rC[:, j, :], pa[:], 0.0, ctx_S[:, j, :],
                        op0=OP.bypass, op1=OP.add, accum_out=stats[:, j:j + 1],
                    )
                    nc.vector.scalar_tensor_tensor(
                        rV[:, j, :], pa[:], 0.0, val_S[:, j, :],
                        op0=OP.bypass, op1=OP.add, accum_out=stats[:, 4 + j:5 + j],
                    )
                    sq = statp.tile([P, D], F32, tag="sq")
                    nc.scalar.activation(
                        sq[:], rC[:, j, :], AF.Square, accum_out=stats[:, 8 + j:9 + j]
                    )
                    sq2 = statp.tile([P, D], F32, tag="sq")
                    nc.scalar.activation(
                        sq2[:], rV[:, j, :], AF.Square, accum_out=stats[:, 12 + j:13 + j]
                    )
                ctx_S = actp.tile([P, NJ, D], F32, tag="ctx_S")
                val_S = actp.tile([P, NJ, D], F32, tag="val_S")
                gc = bc = gv = bv_ = None
                if not ln_identity:
                    gc, bc = wt[f"ln1c g_{i}".replace(" ", "")], wt[f"ln1cb_{i}"]
                    gv, bv_ = wt[f"ln1vg_{i}"], wt[f"ln1vb_{i}"]
                layer_norm(
                    [rC[:, j, :] for j in range(NJ)], stats, 0, NJ,
                    [ctx_S[:, j, :] for j in range(NJ)], gc, bc,
                )
                layer_norm(
                    [rV[:, j, :] for j in range(NJ)], stats, 4, NJ,
                    [val_S[:, j, :] for j in range(NJ)], gv, bv_,
                )
                ctx_T = transpose_S_to_T(ctx_S, "ctx_T")
                val_T = transpose_S_to_T(val_S, "val_T")

                # FFN + residual + LN2c
                hT = bigp.tile([P, NFF, L], WDT, tag="PT")
                f1w = wt[f"f1W_{i}"]
                for m in range(NFF):
                    ph = psq.tile([P, L], F32, tag="mm512")
                    for k in range(NF):
                        nc.tensor.matmul(
                            ph[:], lhsT=_mm(f1w[:, k, m * P:(m + 1) * P]),
                            rhs=_mm(ctx_T[:, k, :]),
                            start=(k == 0), stop=(k == NF - 1),
                        )
                    nc.scalar.activation(
                        hT[:, m, :], ph[:], AF.Relu, bias=wt[f"f1b_{i}"][:, m:m + 1]
                    )
                f2w = wt[f"f2W_{i}"]
                rC = actp.tile([P, NJ, D], F32, tag="rC")
                stats2 = statp.tile([P, 16], F32, tag="stats")
                for j in range(NJ):
                    pf = pss.tile([P, D], F32, tag="psS")
                    for k in range(NFF):
                        nc.tensor.matmul(
                            pf[:], lhsT=_mm(hT[:, k, j * P:(j + 1) * P]),
                            rhs=_mm(f2w[:, k, :]),
                            start=(k == 0), stop=(k == NFF - 1),
                        )
                    if not f2b_zero:
                        nc.vector.tensor_tensor(pf[:], pf[:], wt[f"f2b_{i}"][:], op=OP.add)
                    nc.vector.scalar_tensor_tensor(
                        rC[:, j, :], pf[:], 0.0, ctx_S[:, j, :],
                        op0=OP.bypass, op1=OP.add, accum_out=stats2[:, j:j + 1],
                    )
                    sq = statp.tile([P, D], F32, tag="sq")
                    nc.scalar.activation(
                        sq[:], rC[:, j, :], AF.Square, accum_out=stats2[:, 8 + j:9 + j]
                    )
                ctx_S = actp.tile([P, NJ, D], F32, tag="ctx_S")
                gc = bc = None
                if not ln_identity:
                    gc, bc = wt[f"ln2cg_{i}"], wt[f"ln2cb_{i}"]
                layer_norm(
                    [rC[:, j, :] for j in range(NJ)], stats2, 0, NJ,
                    [ctx_S[:, j, :] for j in range(NJ)], gc, bc,
                )
                ctx_T = transpose_S_to_T(ctx_S, "ctx_T")

            # --- prediction head ---
            sk_S = actp.tile([P, NJ, D], F32, tag="attn_S")
            for j in range(NJ):
                qi = idxp.tile([P, 1], I32, tag="qry")
                nc.sync.dma_start(qi[:], dram["qry_idx"].ap()[b, j * P:(j + 1) * P, :])
                nc.gpsimd.indirect_dma_start(
                    out=sk_S[:, j, :], out_offset=None,
                    in_=dram["skill_emb"].ap(),
                    in_offset=bass.IndirectOffsetOnAxis(ap=qi[:, :1], axis=0),
                )
            sk_T = transpose_S_to_T(sk_S, "attn_T")
            cat_ch = [ctx_T[:, 0, :], ctx_T[:, 1, :], val_T[:, 0, :], val_T[:, 1, :],
                      sk_T[:, 0, :], sk_T[:, 1, :]]
            x1T = bigp.tile([P, NFF, L], WDT, tag="x1T")
            hw1 = wt["head_W1"]
            for m in range(NFF):
                ps = psq.tile([P, L], F32, tag="mm512")
                for k in range(6):
                    nc.tensor.matmul(
                        ps[:], lhsT=_mm(hw1[:, k, m * P:(m + 1) * P]), rhs=_mm(cat_ch[k]),
                        start=(k == 0), stop=(k == 5),
                    )
                nc.scalar.activation(
                    x1T[:, m, :], ps[:], AF.Relu, bias=wt["head_b1"][:, m:m + 1]
                )
            x2T = linT([x1T[:, k, :] for k in range(NFF)], "head_W2", "head_b2",
                       "relu", "x2T", NF)
            pl = psq.tile([P, L], F32, tag="mm512")
            w3 = wt["head_W3"]
            for k in range(NF):
                nc.tensor.matmul(
                    pl[:1, :], lhsT=_mm(w3[:, k, 0:1]), rhs=_mm(x2T[:, k, :]),
                    start=(k == 0), stop=(k == NF - 1),
                )
            lrow = rowp.tile([1, L], F32, tag="row")
            brow = rowp.tile([1, L], F32, tag="row")
            nc.scalar.activation(lrow[:], pl[:1, :], AF.Identity, bias=wt["head_b3"][:, 0:1])
            nc.scalar.activation(brow[:], pl[:1, :], AF.Sigmoid, bias=wt["head_b3"][:, 0:1])
            nc.sync.dma_start(dram["o_logits"].ap()[b:b + 1, :], lrow[:])
            nc.sync.dma_start(dram["o_bce"].ap()[b:b + 1, :], brow[:])

            # --- mastery head ---
            m1T = bigp.tile([P, NFF, L], ADT, tag="x1T")
            m1w = wt["mlp1_W1"]
            for m in range(NFF):
                ps = psq.tile([P, L], F32, tag="mm512")
                for k in range(NF):
                    nc.tensor.matmul(
                        ps[:], lhsT=_mm(m1w[:, k, m * P:(m + 1) * P]),
                        rhs=_mm(ctx_T[:, k, :]),
                        start=(k == 0), stop=(k == NF - 1),
                    )
                nc.scalar.activation(
                    m1T[:, m, :], ps[:], AF.Relu, bias=wt["mlp1_b1"][:, m:m + 1]
                )
            kc_writes = []
            for cc, (c0, cw) in enumerate(CCH):
                w2t = w2p.tile([P, NFF, P], ADT, tag="w2s")
                nc.sync.dma_start(
                    w2t[:, :, :cw],
                    dram["mlp1_W2"].ap().rearrange("(ko p) m -> p ko m", p=P)[:, :, c0:c0 + cw],
                )
                pli = psq.tile([P, L], F32, tag="mm512")
                for k in range(NFF):
                    nc.tensor.matmul(
                        pli[:cw, :], lhsT=_mm(w2t[:, k, :cw]), rhs=_mm(m1T[:, k, :]),
                        start=(k == 0), stop=(k == NFF - 1),
                    )
                inc = kcp.tile([P, L], F32, tag="inc")
                nc.scalar.activation(
                    inc[:cw, :], pli[:cw, :], AF.Exp, bias=wt["b2m2"][:cw, cc:cc + 1]
                )
                kct = kcp.tile([P, L], F32, tag="kct")
                nc.vector.tensor_tensor_scan(
                    kct[:cw, :], inc[:cw, :], ones512[:cw, :], 0.0,
                    op0=OP.add, op1=OP.min,
                )
                wr = nc.sync.dma_start(kc_outs[b].ap()[c0:c0 + cw, :], kct[:cw, :])
                kc_writes.append(wr.ins)

            # ssk = take_along_axis(kc, qry) via flat-index gather from DRAM
            ssk_S = actp.tile([P, NJ], F32, tag="ssk_S")
            kc_flat = kc_outs[b].ap().rearrange("c (l one) -> (c l) one", one=1)
            for j in range(NJ):
                si = idxp.tile([P, 1], I32, tag="ski")
                nc.sync.dma_start(si[:], dram["ssk_idx"].ap()[b, j * P:(j + 1) * P, :])
                g = nc.gpsimd.indirect_dma_start(
                    out=ssk_S[:, j:j + 1], out_offset=None,
                    in_=kc_flat,
                    in_offset=bass.IndirectOffsetOnAxis(ap=si[:, :1], axis=0),
                )
                for w_ in kc_writes:
                    add_dep_helper(g.ins, w_, reason="ssk gather after kc writes")
            psr = psq.tile([P, L], F32, tag="mm512")
            for j in range(NJ):
                nc.tensor.transpose(
                    psr[:1, j * P:(j + 1) * P], ssk_S[:, j:j + 1], identity[:]
                )
            ssk_row = rowp.tile([1, L], WDT, tag="rowr")
            nc.vector.tensor_copy(ssk_row[:], psr[:1, :])

            # mlp2
            y1T = bigp.tile([P, NFF, L], WDT, tag="x1T")
            w1 = wt["mlp2_W1"]
            m2_ch = [ctx_T[:, 0, :], ctx_T[:, 1, :], val_T[:, 0, :], val_T[:, 1, :]]
            for m in range(NFF):
                ps = psq.tile([P, L], F32, tag="mm512")
                for k in range(4):
                    nc.tensor.matmul(
                        ps[:], lhsT=_mm(w1[:, k, m * P:(m + 1) * P]), rhs=_mm(m2_ch[k]),
                        start=(k == 0), stop=False,
                    )
                nc.tensor.matmul(
                    ps[:], lhsT=_mm(wt["mlp2_w1row"][:, m * P:(m + 1) * P]),
                    rhs=_mm(ssk_row[:]), start=False, stop=True,
                )
                nc.scalar.activation(
                    y1T[:, m, :], ps[:], AF.Relu, bias=wt["mlp2_b1"][:, m:m + 1]
                )
            y2T = linT([y1T[:, k, :] for k in range(NFF)], "mlp2_W2", "mlp2_b2",
                       "relu", "x2T", NF)
            pm = psq.tile([P, L], F32, tag="mm512")
            w3m = wt["mlp2_W3"]
            for k in range(NF):
                nc.tensor.matmul(
                    pm[:1, :], lhsT=_mm(w3m[:, k, 0:1]), rhs=_mm(y2T[:, k, :]),
                    start=(k == 0), stop=(k == NF - 1),
                )
            mlrow = rowp.tile([1, L], F32, tag="row")
            mbrow = rowp.tile([1, L], F32, tag="row")
            nc.scalar.activation(mlrow[:], pm[:1, :], AF.Identity, bias=wt["mlp2_b3"][:, 0:1])
            nc.scalar.activation(mbrow[:], pm[:1, :], AF.Sigmoid, bias=wt["mlp2_b3"][:, 0:1])
            nc.sync.dma_start(dram["o_mlogits"].ap()[b:b + 1, :], mlrow[:])
            nc.sync.dma_start(dram["o_mast"].ap()[b:b + 1, :], mbrow[:])

    nc.compile()
    return nc


_PROGRAM_CACHE = {}


def _get_program(flags):
    if flags not in _PROGRAM_CACHE:
        _PROGRAM_CACHE[flags] = _build_program(flags)
    return _PROGRAM_CACHE[flags]


# --------------------------------------------------------------------------
# host-side wrapper
# --------------------------------------------------------------------------

def _col_chunk(v, ncol):
    """[K] bias vector -> [128, ncol] column-chunked layout (pad with 0)."""
    v = np.asarray(v, np.float32)
    out = np.zeros((P, ncol), np.float32)
    for c in range(ncol):
        seg = v[c * P:(c + 1) * P]
        out[: len(seg), c] = seg
    return out


def _prep_inputs(q, r, qry, params):
    """Build the per-core input maps (list of 8 dicts) + build flags."""
    f = np.float32
    q = np.asarray(q).astype(np.int64)
    r = np.asarray(r).astype(np.int64)
    qry = np.asarray(qry).astype(np.int64)
    tok = (q + NUM_C * r).astype(np.int32)          # [B, L]
    qry32 = qry.astype(np.int32)

    pr = {k: np.asarray(v, f) if not isinstance(v, (list, dict)) else v
          for k, v in params.items()}
    blocks = [{k: np.asarray(v, f) for k, v in bp.items()} for bp in pr["blocks"]]

    ln_identity = all(
        np.all(bp[g] == 1.0) and np.all(bp[bn] == 0.0)
        for bp in blocks
        for g, bn in (("ln1c_g", "ln1c_b"), ("ln1v_g", "ln1v_b"), ("ln2c_g", "ln2c_b"))
    )
    bo_eff = [bp["bv"] @ bp["Wo"] + bp["bo"] for bp in blocks]
    bo_zero = all(np.all(x == 0.0) for x in bo_eff)
    f2b_zero = all(np.all(bp["f2_b"] == 0.0) for bp in blocks)
    flags = (ln_identity, bo_zero, f2b_zero)

    common = {}
    pos = np.asarray(pr["pos_emb"], f)               # [512, 256]
    common["pos_S"] = np.ascontiguousarray(
        pos.reshape(NJ, P, D).transpose(1, 0, 2)
    )
    common["ctx_emb"] = np.ascontiguousarray(pr["ctx_emb"])
    common["val_emb"] = np.ascontiguousarray(pr["val_emb"])
    common["skill_emb"] = np.ascontiguousarray(pr["skill_emb"])
    for i, bp in enumerate(blocks):
        common[f"Wq_{i}"] = np.ascontiguousarray(bp["Wq"])
        common[f"Wk_{i}"] = np.ascontiguousarray(bp["Wk"])
        common[f"Wv_{i}"] = np.ascontiguousarray(bp["Wv"])
        common[f"Wo_{i}"] = np.ascontiguousarray(bp["Wo"])
        common[f"f1W_{i}"] = np.ascontiguousarray(bp["f1_W"])
        common[f"f2W_{i}"] = np.ascontiguousarray(bp["f2_W"])
        common[f"bq_{i}"] = _col_chunk(bp["bq"], NF)
        common[f"bk_{i}"] = _col_chunk(bp["bk"], NF)
        common[f"f1b_{i}"] = _col_chunk(bp["f1_b"], NFF)
        if not bo_zero:
            common[f"bo_{i}"] = bo_eff[i].astype(f).reshape(1, D)
        if not f2b_zero:
            common[f"f2b_{i}"] = bp["f2_b"].astype(f).reshape(1, D)
        if not ln_identity:
            for nm, gk, bk_ in (("ln1c", "ln1c_g", "ln1c_b"),
                                ("ln1v", "ln1v_g", "ln1v_b"),
                                ("ln2c", "ln2c_g", "ln2c_b")):
                common[f"{nm}g_{i}"] = bp[gk].astype(f).reshape(1, D)
                common[f"{nm}b_{i}"] = bp[bk_].astype(f).reshape(1, D)
    common["head_W1"] = np.ascontiguousarray(pr["head_W1"])
    common["head_b1"] = _col_chunk(pr["head_b1"], NFF)
    common["head_W2"] = np.ascontiguousarray(pr["head_W2"])
    common["head_b2"] = _col_chunk(pr["head_b2"], NF)
    common["head_W3"] = np.ascontiguousarray(pr["head_W3"])
    common["head_b3"] = np.asarray(pr["head_b3"], f).reshape(1, 1)
    common["mlp1_W1"] = np.ascontiguousarray(pr["mlp1_W1"])
    common["mlp1_b1"] = _col_chunk(pr["mlp1_b1"], NFF)
    common["mlp1_W2"] = np.ascontiguousarray(pr["mlp1_W2"])
    common["b2m2"] = _col_chunk(
        np.asarray(pr["mlp1_b2"], f) + float(np.asarray(pr["inc_scale"], f)), NCC
    )
    common["mlp2_W1"] = np.ascontiguousarray(pr["mlp2_W1"])
    common["mlp2_b1"] = _col_chunk(pr["mlp2_b1"], NFF)
    common["mlp2_W2"] = np.ascontiguousarray(pr["mlp2_W2"])
    common["mlp2_b2"] = _col_chunk(pr["mlp2_b2"], NF)
    common["mlp2_W3"] = np.ascontiguousarray(pr["mlp2_W3"])
    common["mlp2_b3"] = np.asarray(pr["mlp2_b3"], f).reshape(1, 1)

    lidx = np.arange(L, dtype=np.int32)
    in_maps = []
    for c in range(NCORES):
        m = dict(common)
        tb = tok[c * BL:(c + 1) * BL]                # [BL, L]
        qb = qry32[c * BL:(c + 1) * BL]
        m["tok_idx"] = np.ascontiguousarray(tb[:, :, None])
        m["qry_idx"] = np.ascontiguousarray(qb[:, :, None])
        m["ssk_idx"] = np.ascontiguousarray(
            (qb * L + lidx[None, :]).astype(np.int32)[:, :, None]
        )
        in_maps.append(m)
    return in_maps, flags


def kernel(q, r, qry, params):
    in_maps, flags = _prep_inputs(q, r, qry, params)
    nc = _get_program(flags)
    res = bass_utils.run_bass_kernel_spmd(
        nc, in_maps, core_ids=list(range(NCORES))
    )
    outs = res.results
    bce = np.concatenate([outs[c]["o_bce"] for c in range(NCORES)], 0)
    logits = np.concatenate([outs[c]["o_logits"] for c in range(NCORES)], 0)
    mast = np.concatenate([outs[c]["o_mast"] for c in range(NCORES)], 0)
    mlog = np.concatenate([outs[c]["o_mlogits"] for c in range(NCORES)], 0)
    kc = np.empty((B, L, NUM_C), np.float32)
    for c in range(NCORES):
        for b in range(BL):
            kc[c * BL + b] = outs[c][f"o_kc_{b}"].T
    return (bce, mast, kc, logits, mlog)
